# revision 1
# baseline (speedup 1.0000x reference)
"""Trainium2 Bass kernel for nn_CutLayer (histogram_binning).

Strategy (data-parallel over events, 8 cores):
  L1: per-core min/max of the feature column (device reduce).
  L2: per-core exact per-edge cumulative counts, split across two engines:
      - Vector (DVE): scalar_tensor_tensor (x <= e) * w with w = 1 + 4096*y,
        per-partition accumulated over 3906-element halves, packing
        count and signal-count into one exact fp32 integer.
      - Scalar (ACT): sign(x - e) with per-partition accumulation on both
        the full stream and a signal-masked stream; counts recovered as
        (N + ties - sum_sign) / 2 (exact +-1 sums).
  host: combine counts, repair lt/le tie counts from a tiny candidate set,
      replicate the reference's tiny E^2 pair search bit-exactly with
      eager CPU jax, producing (lower, upper, case).
  L3: per-core case-specialized predicate (4 lazily-built programs; only
      the dispatched case compiles): cases 0/1 are a single 2x-rate
      tensor_scalar compare; cases 2/3 are one compare plus one fused
      scalar_tensor_tensor combine. All compares exact.

Events per core: 1_000_000; the device handles 128*7812 = 999_936 of them
(SBUF tile [128, 7812]); the 64-per-core remainder is handled exactly on
the host (512 events total).
"""

from contextlib import ExitStack

import numpy as np

import concourse.bass as bass
import concourse.mybir as mybir
from concourse.bass_utils import run_bass_kernel_spmd

N = 8_000_000
N_CORES = 8
CORE_N = N // N_CORES            # 1_000_000
P = 128
F = 7812                         # free-dim columns per partition
H = F // 2                       # packed-accum half (counts < 4096)
DEV_N = P * F                    # 999_936 device events per core
N_DEV_TOT = DEV_N * N_CORES      # 7_999_488
N_BINS = 50
E = N_BINS + 1                   # 51 edges
EPS = 1e-7
KD = 32                          # edges handled by the vector engine
KA = E - KD                      # edges handled by the scalar engine
PACK = 4096.0                    # signal-count multiplier (exact < 2^24)

FP32 = mybir.dt.float32
BF16 = mybir.dt.bfloat16
I32 = mybir.dt.int32
AX = mybir.AxisListType
OP = mybir.AluOpType
ACT = mybir.ActivationFunctionType

CORE_IDS = list(range(N_CORES))


# --------------------------------------------------------------------------
# Bass programs (built once per process)
# --------------------------------------------------------------------------

def _build_minmax():
    nc = bass.Bass()
    x = nc.declare_dram_parameter("x", [DEV_N], FP32, isOutput=False)
    mn = nc.declare_dram_parameter("mn", [P], FP32, isOutput=True)
    mx = nc.declare_dram_parameter("mx", [P], FP32, isOutput=True)
    with (
        nc.sbuf_tensor([P, F], FP32) as xt,
        nc.sbuf_tensor([P, 2], FP32) as acc,
        nc.semaphore() as dsem,
        nc.semaphore() as csem,
        nc.Block() as block,
    ):
        @block.sync
        def _(sync):
            sync.dma_start(xt[:], x[:].rearrange("(p f) -> p f", p=P)).then_inc(
                dsem, 16
            )
            sync.wait_ge(csem, 2)
            sync.dma_start(mn[:], acc[:, 0:1]).then_inc(dsem, 16)
            sync.dma_start(mx[:], acc[:, 1:2]).then_inc(dsem, 16)
            sync.wait_ge(dsem, 48)

        @block.vector
        def _(vector):
            vector.wait_ge(dsem, 16)
            vector.tensor_reduce(acc[:, 0:1], xt[:], axis=AX.X, op=OP.min).then_inc(
                csem, 1
            )
            vector.tensor_reduce(acc[:, 1:2], xt[:], axis=AX.X, op=OP.max).then_inc(
                csem, 1
            )
    return nc


def _build_counts():
    nc = bass.Bass()
    x = nc.declare_dram_parameter("x", [DEV_N], FP32, isOutput=False)
    w = nc.declare_dram_parameter("w", [DEV_N], FP32, isOutput=False)
    xs = nc.declare_dram_parameter("xs", [DEV_N], FP32, isOutput=False)
    ed = nc.declare_dram_parameter("edges", [P, 2 * E], FP32, isOutput=False)
    opk = nc.declare_dram_parameter("acc_pk", [P, 2 * KD], FP32, isOutput=True)
    osa = nc.declare_dram_parameter("acc_sa", [P, KA], FP32, isOutput=True)
    oss = nc.declare_dram_parameter("acc_ss", [P, KA], FP32, isOutput=True)
    with ExitStack() as es:
        ec = es.enter_context
        xt = ec(nc.sbuf_tensor([P, F], FP32))
        wt = ec(nc.sbuf_tensor([P, F], FP32))
        xst = ec(nc.sbuf_tensor([P, F], FP32))
        scr = ec(nc.sbuf_tensor([P, F], FP32))
        asca = ec(nc.sbuf_tensor([P, F], BF16))
        ascb = ec(nc.sbuf_tensor([P, F], BF16))
        edt = ec(nc.sbuf_tensor([P, 2 * E], FP32))
        apk = ec(nc.sbuf_tensor([P, 2 * KD], FP32))
        asa = ec(nc.sbuf_tensor([P, KA], FP32))
        ass = ec(nc.sbuf_tensor([P, KA], FP32))
        names = ["dse", "dsx", "dsx1", "dsw", "dsw1", "dss", "dso",
                 "csem", "v0", "v1", "t0", "t1"]
        dse, dsx, dsx1, dsw, dsw1, dss, dso, csem, v0, v1, t0, t1 = (
            ec(nc.semaphore(n)) for n in names
        )
        block = ec(nc.Block())
        @block.sync
        def _(sync):
            xv = x[:].rearrange("(p f) -> p f", p=P)
            wv = w[:].rearrange("(p f) -> p f", p=P)
            sync.dma_start(edt[:], ed[:]).then_inc(dse, 16)
            # interleave x/w halves so the vector engine can start on the
            # first data half as soon as possible
            sync.dma_start(xt[:, 0:H], xv[:, 0:H]).then_inc(dsx, 16)
            sync.dma_start(wt[:, 0:H], wv[:, 0:H]).then_inc(dsw, 16)
            sync.dma_start(xt[:, H:F], xv[:, H:F]).then_inc(dsx1, 16)
            sync.dma_start(wt[:, H:F], wv[:, H:F]).then_inc(dsw1, 16)
            sync.dma_start(xst[:], xs[:].rearrange("(p f) -> p f", p=P)).then_inc(
                dss, 16
            )
            # ACT typically retires first: ship its accumulators while the
            # vector engine finishes, then the packed accumulators.
            sync.wait_ge(t0, 2 * ((KA + 1) // 2))
            sync.wait_ge(t1, 2 * (KA // 2))
            sync.dma_start(osa[:], asa[:]).then_inc(dso, 16)
            sync.dma_start(oss[:], ass[:]).then_inc(dso, 16)
            sync.wait_ge(v0, KD)
            sync.wait_ge(v1, KD)
            sync.dma_start(opk[:], apk[:]).then_inc(dso, 16)
            sync.wait_ge(dso, 48)

        @block.vector
        def _(vector):
            # phase 1: all edges on data half 0 (needs edges + x0 + w0),
            # phase 2: all edges on data half 1 — accumulator slots are
            # per (edge, half) so order is free. Scratch regions ping-pong
            # by instruction parity with retirement semaphores.
            vector.wait_ge(dse, 16)
            vector.wait_ge(dsx, 16)
            vector.wait_ge(dsw, 16)
            nh = [0, 0]  # completed instruction count per scratch region
            ninstr = 0
            for dh in range(2):
                hs = dh * H
                if dh == 1:
                    vector.wait_ge(dsx1, 16)
                    vector.wait_ge(dsw1, 16)
                for i in range(KD):
                    e = edt[:, i : i + 1]
                    rr = ninstr % 2
                    if nh[rr] >= 1:
                        vector.wait_ge([v0, v1][rr], nh[rr])
                    vector.scalar_tensor_tensor(
                        scr[:, rr * H : rr * H + H],
                        xt[:, hs : hs + H],
                        e,
                        wt[:, hs : hs + H],
                        op0=OP.is_le,
                        op1=OP.mult,
                        accum_out=apk[:, 2 * i + dh : 2 * i + dh + 1],
                    ).then_inc([v0, v1][rr], 1)
                    nh[rr] += 1
                    ninstr += 1

        @block.scalar
        def _(scalar):
            # x-stream first (needs edges + x only)
            scalar.wait_ge(dse, 16)
            scalar.wait_ge(dsx, 16)
            scalar.wait_ge(dsx1, 16)
            na = [0, 0]
            scrs = [asca, ascb]
            sems = [t0, t1]
            for i in range(KA):
                ne = edt[:, E + KD + i : E + KD + i + 1]  # negated edge
                hh = i % 2
                if na[hh] >= 1:
                    scalar.wait_ge(sems[hh], na[hh])
                scalar.activation(
                    scrs[hh][:], xt[:], ACT.Sign, bias=ne, scale=1.0,
                    accum_out=asa[:, i : i + 1],
                ).then_inc(sems[hh], 1)
                na[hh] += 1
            # signal stream (needs xs)
            scalar.wait_ge(dss, 16)
            for i in range(KA):
                ne = edt[:, E + KD + i : E + KD + i + 1]
                hh = i % 2
                if na[hh] >= 1:
                    scalar.wait_ge(sems[hh], na[hh])
                scalar.activation(
                    scrs[hh][:], xst[:], ACT.Sign, bias=ne, scale=1.0,
                    accum_out=ass[:, i : i + 1],
                ).then_inc(sems[hh], 1)
                na[hh] += 1
    return nc


def _build_pred(case: int):
    """Case-specialized predicate:
    0: x <= lo                    (1 pass)
    1: x >= lo                    (1 pass)
    2: (x >= lo) & (x <= up)      (2 passes)
    3: (x <= lo) | (x >= up)      (2 passes, disjoint -> add)
    """
    nc = bass.Bass()
    x = nc.declare_dram_parameter("x", [DEV_N], FP32, isOutput=False)
    pr = nc.declare_dram_parameter("prm", [P, 8], FP32, isOutput=False)
    out = nc.declare_dram_parameter("pred", [DEV_N], I32, isOutput=True)
    with (
        nc.sbuf_tensor([P, F], FP32) as xt,
        nc.sbuf_tensor([P, F], FP32) as t,
        nc.sbuf_tensor([P, F], I32) as pi,
        nc.sbuf_tensor([P, 8], FP32) as prm,
        nc.semaphore() as dsem,
        nc.semaphore() as csem,
        nc.semaphore() as tsem,
        nc.Block() as block,
    ):
        @block.sync
        def _(sync):
            sync.dma_start(prm[:], pr[:]).then_inc(dsem, 16)
            sync.dma_start(xt[:], x[:].rearrange("(p f) -> p f", p=P)).then_inc(
                dsem, 16
            )
            sync.wait_ge(csem, 1)
            sync.dma_start(
                out[:].rearrange("(p f) -> p f", p=P), pi[:]
            ).then_inc(dsem, 16)
            sync.wait_ge(dsem, 48)

        @block.vector
        def _(vector):
            vector.wait_ge(dsem, 32)
            lo = prm[:, 0:1]
            up = prm[:, 1:2]
            if case == 0:
                vector.tensor_scalar(pi[:], xt[:], lo, None, OP.is_le).then_inc(
                    csem, 1
                )
            elif case == 1:
                vector.tensor_scalar(pi[:], xt[:], lo, None, OP.is_ge).then_inc(
                    csem, 1
                )
            elif case == 2:
                vector.tensor_scalar(t[:], xt[:], up, None, OP.is_le).then_inc(
                    tsem, 1
                )
                vector.wait_ge(tsem, 1)
                vector.scalar_tensor_tensor(
                    pi[:], xt[:], lo, t[:], op0=OP.is_ge, op1=OP.mult
                ).then_inc(csem, 1)
            else:
                vector.tensor_scalar(t[:], xt[:], up, None, OP.is_ge).then_inc(
                    tsem, 1
                )
                vector.wait_ge(tsem, 1)
                vector.scalar_tensor_tensor(
                    pi[:], xt[:], lo, t[:], op0=OP.is_le, op1=OP.add
                ).then_inc(csem, 1)
    return nc


_PROGRAMS: dict = {}


def _prog(name):
    if name not in _PROGRAMS:
        if name.startswith("pred"):
            _PROGRAMS[name] = _build_pred(int(name[4:]))
        else:
            _PROGRAMS[name] = {
                "minmax": _build_minmax,
                "counts": _build_counts,
            }[name]()
    return _PROGRAMS[name]


# --------------------------------------------------------------------------
# Host orchestration
# --------------------------------------------------------------------------

LAST_EXEC_NS: list = []


_CACHE_SET = False


def _enable_jit_cache():
    # Persist compiled executables (which embed the NEFF) across processes;
    # harmless no-op if the backend doesn't support serialization.
    global _CACHE_SET
    if _CACHE_SET:
        return
    _CACHE_SET = True
    try:
        import jax

        jax.config.update("jax_compilation_cache_dir", "/tmp/jax_bass_cache")
        jax.config.update("jax_persistent_cache_min_compile_time_secs", 1.0)
        jax.config.update("jax_persistent_cache_min_entry_size_bytes", 0)
    except Exception:
        pass


def _run(name, in_maps):
    import os

    _enable_jit_cache()
    trace = bool(int(os.environ.get("BASS_KERNEL_PROFILE", "0")))
    r = run_bass_kernel_spmd(_prog(name), in_maps, CORE_IDS, trace=trace)
    if trace:
        LAST_EXEC_NS.append((name, r.exec_time_ns, r.mean_exec_time_ns))
    return r.results


def _dev_shard(arr, c):
    return arr[c * CORE_N : c * CORE_N + DEV_N]


def _tail_shard(arr, c):
    return arr[c * CORE_N + DEV_N : (c + 1) * CORE_N]


def kernel(inputs: np.ndarray, targets: np.ndarray) -> np.ndarray:
    x_full = np.ascontiguousarray(inputs[:, 0]).astype(np.float32, copy=False)
    y_full = np.asarray(targets)

    tails_x = [_tail_shard(x_full, c) for c in CORE_IDS]
    tails_y = [_tail_shard(y_full, c) for c in CORE_IDS]
    tail_x = np.concatenate(tails_x)
    tail_y = np.concatenate(tails_y)

    # ---- L1: global min/max -------------------------------------------------
    LAST_EXEC_NS.clear()
    res1 = _run("minmax", [{"x": _dev_shard(x_full, c)} for c in CORE_IDS])
    gmin = np.float32(min(min(r["mn"].min() for r in res1), tail_x.min()))
    gmax = np.float32(max(max(r["mx"].max() for r in res1), tail_x.max()))

    # ---- edges: replicate jnp.linspace bit-exactly (eager CPU jax) ----------
    import jax
    import jax.numpy as jnp

    cpu = jax.devices("cpu")[0]
    with jax.default_device(cpu):
        edges = np.asarray(jnp.linspace(jnp.float32(gmin), jnp.float32(gmax), E))

    # ---- L2: per-edge counts ------------------------------------------------
    sig_mask = y_full == 1
    # Finite sentinel above every possible edge (sim paths reject inf inputs).
    sent = np.float32(np.finfo(np.float32).max)
    x_sig = np.where(sig_mask, x_full, sent).astype(np.float32)
    w_full = (1.0 + PACK * sig_mask).astype(np.float32)
    ed_in = np.concatenate([edges, -edges]).astype(np.float32)
    edges_rep = np.ascontiguousarray(np.broadcast_to(ed_in, (P, 2 * E)))

    res2 = _run(
        "counts",
        [
            {
                "x": _dev_shard(x_full, c),
                "w": _dev_shard(w_full, c),
                "xs": _dev_shard(x_sig, c),
                "edges": edges_rep,
            }
            for c in CORE_IDS
        ],
    )

    # ---- exact tie counts (x == edge) from a tiny candidate set -------------
    h = (np.float32(gmax) - np.float32(gmin)) / np.float32(N_BINS)
    inv_h = np.float32(1.0) / h if h != 0 else np.float32(0.0)
    u = (x_full - gmin) * inv_h
    r_near = np.rint(u)
    cand = np.abs(u - r_near) < np.float32(0.01)
    idx = np.flatnonzero(cand)
    T_all = np.zeros(E, np.float64)
    Tsig_all = np.zeros(E, np.float64)
    T_dev = np.zeros(E, np.float64)
    Tsig_dev = np.zeros(E, np.float64)
    if idx.size:
        kn = np.clip(r_near[idx].astype(np.int64), 0, E - 1)
        is_tie = x_full[idx] == edges[kn]
        tidx = idx[is_tie]
        tie_k = kn[is_tie]
        tie_sig = sig_mask[tidx]
        tie_dev = (tidx % CORE_N) < DEV_N
        np.add.at(T_all, tie_k, 1.0)
        np.add.at(Tsig_all, tie_k[tie_sig], 1.0)
        np.add.at(T_dev, tie_k[tie_dev], 1.0)
        np.add.at(Tsig_dev, tie_k[tie_dev & tie_sig], 1.0)

    # ---- decode device counts ----------------------------------------------
    cnt_le = np.zeros(E, np.float64)
    sig_le = np.zeros(E, np.float64)
    cnt_pk = np.zeros(2 * KD, np.int64)
    sig_pk = np.zeros(2 * KD, np.int64)
    sa = np.zeros(KA, np.float64)
    ss = np.zeros(KA, np.float64)
    for r in res2:
        # decode per accumulator slot (each packs cnt<4096 with 4096*sig)
        a = r["acc_pk"].astype(np.int64)
        s_part = a // int(PACK)
        c_part = a - int(PACK) * s_part
        cnt_pk += c_part.sum(axis=0)
        sig_pk += s_part.sum(axis=0)
        sa += r["acc_sa"].astype(np.float64).sum(axis=0)
        ss += r["acc_ss"].astype(np.float64).sum(axis=0)
    cnt_le[:KD] = cnt_pk.reshape(KD, 2).sum(axis=1)
    sig_le[:KD] = sig_pk.reshape(KD, 2).sum(axis=1)
    cnt_le[KD:] = (N_DEV_TOT + T_dev[KD:] - sa) / 2.0
    sig_le[KD:] = (N_DEV_TOT + Tsig_dev[KD:] - ss) / 2.0

    # tail events, exact
    cnt_le += (tail_x[:, None] <= edges[None, :]).sum(axis=0)
    sig_le += (tail_x[tail_y == 1][:, None] <= edges[None, :]).sum(axis=0)

    cnt_lt = cnt_le - T_all
    sig_lt = sig_le - Tsig_all

    ns_le = sig_le.astype(np.float32)
    ns_lt = sig_lt.astype(np.float32)
    nb_le = (cnt_le - sig_le).astype(np.float32)
    nb_lt = (cnt_lt - sig_lt).astype(np.float32)

    # ---- replicate the reference's tiny pair search (eager CPU jax) ---------
    with jax.default_device(cpu):
        ns_le_j = jnp.asarray(ns_le)
        ns_lt_j = jnp.asarray(ns_lt)
        nb_le_j = jnp.asarray(nb_le)
        nb_lt_j = jnp.asarray(nb_lt)
        n_f = jnp.float32(N)
        Ns = ns_le_j[-1]
        Nb = n_f - Ns

        hist0 = nb_le_j[1:] - nb_lt_j[:-1]
        hist1 = ns_le_j[1:] - ns_lt_j[:-1]

        gt0 = hist0 > hist1
        cand0 = jnp.logical_xor(gt0[:-1], gt0[1:]) & (hist0[:-1] > 0)
        gt1 = hist1 > hist0
        cand1 = jnp.logical_xor(gt1[:-1], gt1[1:]) & (hist1[:-1] > 0)
        mask = jnp.zeros((E,), bool).at[1:N_BINS].set(cand0 | cand1)
        cnt = jnp.sum(mask)
        mask = mask.at[-1].set(mask[-1] | (cnt == 1))

        a_c = -jnp.log1p(jnp.float32(-EPS))
        b_c = -jnp.log(jnp.float32(EPS))

        def bce(correct):
            return ((n_f - correct) * b_c + correct * a_c) / n_f

        c0 = ns_le_j + (Nb - nb_le_j)
        c1 = (Ns - ns_lt_j) + nb_lt_j
        c2 = (ns_le_j[None, :] - ns_lt_j[:, None]) + Nb - (
            nb_le_j[None, :] - nb_lt_j[:, None]
        )
        c3 = ns_le_j[:, None] + (Ns - ns_lt_j[None, :]) + (
            nb_le_j[None, :] - nb_lt_j[:, None]
        )

        L = jnp.stack(
            [
                jnp.broadcast_to(bce(c0)[:, None], (E, E)),
                jnp.broadcast_to(bce(c1)[:, None], (E, E)),
                bce(c2),
                bce(c3),
            ]
        )
        per_pair_min = jnp.min(L, axis=0)
        per_pair_case = jnp.argmin(L, axis=0)

        idxs = jnp.arange(E)
        valid = mask[:, None] & mask[None, :] & (idxs[:, None] < idxs[None, :])
        flat = jnp.argmin(jnp.where(valid, per_pair_min, jnp.inf))
        i = int(flat) // E
        j = int(flat) % E
        lower = np.float32(edges[i])
        upper = np.float32(edges[j])
        case = int(per_pair_case[i, j])

    # ---- L3: predicate (case-specialized program; exact compares) --------
    prm = np.zeros((P, 8), np.float32)
    prm[:, 0] = lower
    prm[:, 1] = upper

    res3 = _run(
        f"pred{case}", [{"x": _dev_shard(x_full, c), "prm": prm} for c in CORE_IDS]
    )

    out = np.empty(N, np.int32)
    for c in CORE_IDS:
        out[c * CORE_N : c * CORE_N + DEV_N] = res3[c]["pred"]
        tx = tails_x[c]
        if case == 0:
            tp = tx <= lower
        elif case == 1:
            tp = tx >= lower
        elif case == 2:
            tp = (tx >= lower) & (tx <= upper)
        else:
            tp = (tx <= lower) | (tx >= upper)
        out[c * CORE_N + DEV_N : (c + 1) * CORE_N] = tp.astype(np.int32)
    return out



# revision 4
# speedup vs baseline: 1.9768x; 1.9768x over previous
"""Trainium2 Bass kernel for nn_CutLayer (histogram_binning).

Two device launches over 8 cores (data-parallel on events):

L1 "counts": per-core class-compacted bf16 tile [128, F2] (rows 0-63 =
  signal events, rows 64-127 = background, pads = +BIG). 51 bf16-grid
  edge thresholds are counted by three engine paths running concurrently:
    - PE path (N_B edges): DVE plain tensor_scalar compare at 4x rate
      (bf16) into ping-pong buffers; PE reduces each compare tile with a
      one-hot stationary matmul into PSUM rows [2 per edge], accumulated
      across 512-column chunks; one final DVE reduce -> [2*N_B, 1].
    - DVE-accum path (N_A edges): fused tensor_scalar compare+accum (1x).
    - ACT path (N_C edges): Sign activation with bias strictly between
      bf16 grid points (no sign(0) ties) + accumulator.
  Host converts to exact fp32-semantics counts (le and lt) by correcting
  a small candidate set of events within a few bf16 ulps of each edge,
  then replicates the reference's pair search bit-exactly (eager CPU jax)
  to produce (lower, upper, case).

L2 "pred": case-specialized bf16 compare(s) on the original-order
  events; host flips the few events within bf16 rounding of the chosen
  cuts and handles the 512-event layout tail exactly.

Host work is O(N) numpy prep/fixup only: min/max, bf16 casts, class
compaction, candidate repair, tiny 51x51 pair search.
"""

from contextlib import ExitStack

import numpy as np
import ml_dtypes

import concourse.bass as bass
import concourse.mybir as mybir
from concourse.bass_utils import run_bass_kernel_spmd

N = 8_000_000
N_CORES = 8
CORE_N = N // N_CORES            # 1_000_000
P = 128
HP = 64                          # rows per class in the counts tile
N_BINS = 50
E = N_BINS + 1                   # 51 edges
EPS = 1e-7
BIG = np.float32(1.0e30)         # bf16-exact sentinel above every edge

# pred layout (original order)
FP = 7812
DEV_N = P * FP                   # 999_936 device events/core for pred

# counts engine split (sums to E)
N_B = 26                         # DVE compare -> PE matmul reduce
N_A = 7                          # DVE fused compare+accum
N_C = 18                         # ACT sign path
CH = 512                         # psum chunk columns

FP32 = mybir.dt.float32
BF16 = mybir.dt.bfloat16
I32 = mybir.dt.int32
AX = mybir.AxisListType
OP = mybir.AluOpType
ACT = mybir.ActivationFunctionType

CORE_IDS = list(range(N_CORES))
BF = ml_dtypes.bfloat16


# --------------------------------------------------------------------------
# Bass programs
# --------------------------------------------------------------------------

def _build_counts_v2(F2: int):
    nchunks = (F2 + CH - 1) // CH
    MROWS = 2 * N_B
    nc = bass.Bass()
    xt_in = nc.declare_dram_parameter("xt", [P, F2], BF16, isOutput=False)
    ed = nc.declare_dram_parameter("edges", [P, 2 * E], FP32, isOutput=False)
    oh_in = nc.declare_dram_parameter("oh", [P, N_B * MROWS], BF16,
                                      isOutput=False)
    acca_o = nc.declare_dram_parameter("acca", [P, N_A], FP32, isOutput=True)
    accc_o = nc.declare_dram_parameter("accc", [P, N_C], FP32, isOutput=True)
    accb_o = nc.declare_dram_parameter("accb", [MROWS, 1], FP32, isOutput=True)
    with ExitStack() as es:
        ec = es.enter_context
        xt = ec(nc.sbuf_tensor([P, F2], BF16))
        cb = [ec(nc.sbuf_tensor(f"cb{i}", [P, F2], BF16)) for i in range(2)]
        scra = ec(nc.sbuf_tensor([P, F2], BF16))
        scrc = ec(nc.sbuf_tensor([P, F2], BF16))
        edt = ec(nc.sbuf_tensor([P, 2 * E], FP32))
        oht = ec(nc.sbuf_tensor([P, N_B * MROWS], BF16))
        acca = ec(nc.sbuf_tensor([P, N_A], FP32))
        accc = ec(nc.sbuf_tensor([P, N_C], FP32))
        accb = ec(nc.sbuf_tensor([MROWS, 1], FP32))
        ps = ec(nc.psum_tensor([MROWS, CH], FP32))
        dse = ec(nc.semaphore("dse"))
        dsx0 = ec(nc.semaphore("dsx0"))
        dsx1 = ec(nc.semaphore("dsx1"))
        vprod = ec(nc.semaphore("vprod"))
        tcons = ec(nc.semaphore("tcons"))
        adone = ec(nc.semaphore("adone"))
        cdone = ec(nc.semaphore("cdone"))
        bdone = ec(nc.semaphore("bdone"))
        dso = ec(nc.semaphore("dso"))
        block = ec(nc.Block())

        H2 = F2 // 2

        @block.sync
        def _(sync):
            sync.dma_start(edt[:], ed[:]).then_inc(dse, 16)
            sync.dma_start(oht[:], oh_in[:]).then_inc(dse, 16)
            sync.dma_start(xt[:, 0:H2], xt_in[:, 0:H2]).then_inc(dsx0, 16)
            sync.wait_ge(adone, 1)
            sync.dma_start(acca_o[:], acca[:]).then_inc(dso, 16)
            sync.wait_ge(cdone, 1)
            sync.dma_start(accc_o[:], accc[:]).then_inc(dso, 16)
            sync.wait_ge(bdone, 1)
            sync.dma_start(accb_o[:], accb[:]).then_inc(dso, 16)
            sync.wait_ge(dso, 48)

        @block.scalar
        def _(scalar):
            scalar.dma_start(xt[:, H2:F2], xt_in[:, H2:F2]).then_inc(dsx1, 16)
            scalar.wait_ge(dse, 32)
            scalar.wait_ge(dsx0, 16)
            scalar.wait_ge(dsx1, 16)
            for j in range(N_C):
                ins = scalar.activation(
                    scrc[:], xt[:], ACT.Sign, bias=edt[:, E + j : E + j + 1],
                    scale=1.0, accum_out=accc[:, j : j + 1],
                )
                if j == N_C - 1:
                    ins.then_inc(cdone, 1)

        @block.vector
        def _(vector):
            vector.wait_ge(dse, 32)
            vector.wait_ge(dsx0, 16)
            vector.wait_ge(dsx1, 16)
            na = 0
            last_a_ins = None
            for i in range(N_B):
                if i >= 2:
                    vector.wait_ge(tcons, i - 1)
                vector.tensor_scalar(
                    cb[i % 2][:], xt[:], edt[:, N_A + i : N_A + i + 1], None,
                    OP.is_le,
                ).then_inc(vprod, 1)
                if (i % 4 == 3) and na < N_A:
                    last_a_ins = vector.tensor_scalar(
                        scra[:], xt[:], edt[:, na : na + 1], 0.0,
                        OP.is_le, OP.add, accum_out=acca[:, na : na + 1],
                    )
                    na += 1
            while na < N_A:
                last_a_ins = vector.tensor_scalar(
                    scra[:], xt[:], edt[:, na : na + 1], 0.0,
                    OP.is_le, OP.add, accum_out=acca[:, na : na + 1],
                )
                na += 1
            if last_a_ins is not None:
                last_a_ins.then_inc(adone, 1)
            vector.wait_ge(tcons, N_B)
            vector.tensor_reduce(
                accb[:, 0:1], ps[:, 0:CH], axis=AX.X, op=OP.add
            ).then_inc(bdone, 1)

        @block.tensor
        def _(tensor):
            first = True
            for i in range(N_B):
                tensor.wait_ge(vprod, i + 1)
                w = oht[:, i * MROWS : (i + 1) * MROWS]
                for c in range(nchunks):
                    c0 = c * CH
                    c1 = min(F2, c0 + CH)
                    ins = tensor.matmul(
                        ps[:, 0 : c1 - c0], w, cb[i % 2][:, c0:c1],
                        start=first,
                        stop=(i == N_B - 1 and c == nchunks - 1),
                        skip_group_check=True,
                    )
                    first = False
                ins.then_inc(tcons, 1)
    return nc


def _build_pred(case: int):
    """Case-specialized predicate on bf16 events (original order):
    0: x <= lo ; 1: x >= lo ; 2: (x >= lo) & (x <= up) ;
    3: (x <= lo) | (x >= up)  (disjoint -> add)
    """
    nc = bass.Bass()
    x = nc.declare_dram_parameter("x", [DEV_N], BF16, isOutput=False)
    pr = nc.declare_dram_parameter("prm", [P, 8], FP32, isOutput=False)
    out = nc.declare_dram_parameter("pred", [DEV_N], BF16, isOutput=True)
    HF = FP // 2
    with (
        nc.sbuf_tensor([P, FP], BF16) as xt,
        nc.sbuf_tensor([P, FP], BF16) as t,
        nc.sbuf_tensor([P, FP], BF16) as s,
        nc.sbuf_tensor([P, FP], BF16) as pi,
        nc.sbuf_tensor([P, 8], FP32) as prm,
        nc.semaphore("d0") as d0,
        nc.semaphore("d1") as d1,
        nc.semaphore("csem") as csem,
        nc.semaphore("dso") as dso,
        nc.Block() as block,
    ):
        xv = x[:].rearrange("(p f) -> p f", p=P)
        ov = out[:].rearrange("(p f) -> p f", p=P)

        @block.sync
        def _(sync):
            sync.dma_start(prm[:], pr[:]).then_inc(d0, 16)
            sync.dma_start(xt[:, 0:HF], xv[:, 0:HF]).then_inc(d0, 16)
            sync.wait_ge(csem, 1)
            sync.dma_start(ov[:, 0:HF], pi[:, 0:HF]).then_inc(dso, 16)
            sync.wait_ge(csem, 2)
            sync.dma_start(ov[:, HF:FP], pi[:, HF:FP]).then_inc(dso, 16)
            sync.wait_ge(dso, 32)

        @block.scalar
        def _(scalar):
            scalar.dma_start(xt[:, HF:FP], xv[:, HF:FP]).then_inc(d1, 16)

        @block.vector
        def _(vector):
            lo = prm[:, 0:1]
            up = prm[:, 1:2]
            vector.wait_ge(d0, 32)
            for h, sem in ((0, d0), (1, d1)):
                if h == 1:
                    vector.wait_ge(d1, 16)
                sl = slice(0, HF) if h == 0 else slice(HF, FP)
                if case == 0:
                    vector.tensor_scalar(
                        pi[:, sl], xt[:, sl], lo, None, OP.is_le
                    ).then_inc(csem, 1)
                elif case == 1:
                    vector.tensor_scalar(
                        pi[:, sl], xt[:, sl], lo, None, OP.is_ge
                    ).then_inc(csem, 1)
                elif case == 2:
                    vector.tensor_scalar(t[:, sl], xt[:, sl], up, None,
                                         OP.is_le)
                    vector.tensor_scalar(s[:, sl], xt[:, sl], lo, None,
                                         OP.is_ge)
                    vector.tensor_tensor(
                        pi[:, sl], s[:, sl], t[:, sl], OP.mult
                    ).then_inc(csem, 1)
                else:
                    vector.tensor_scalar(t[:, sl], xt[:, sl], up, None,
                                         OP.is_ge)
                    vector.tensor_scalar(s[:, sl], xt[:, sl], lo, None,
                                         OP.is_le)
                    vector.tensor_tensor(
                        pi[:, sl], s[:, sl], t[:, sl], OP.add
                    ).then_inc(csem, 1)
    return nc


_PROGRAMS: dict = {}


def _prog(name, *args):
    key = (name, args)
    if key not in _PROGRAMS:
        if name == "counts":
            _PROGRAMS[key] = _build_counts_v2(*args)
        else:
            _PROGRAMS[key] = _build_pred(int(name[4:]))
    return _PROGRAMS[key]


LAST_EXEC_NS: list = []
_CACHE_SET = False


def _enable_jit_cache():
    global _CACHE_SET
    if _CACHE_SET:
        return
    _CACHE_SET = True
    try:
        import jax

        jax.config.update("jax_compilation_cache_dir", "/tmp/jax_bass_cache")
        jax.config.update("jax_persistent_cache_min_compile_time_secs", 1.0)
        jax.config.update("jax_persistent_cache_min_entry_size_bytes", 0)
    except Exception:
        pass


def _run(name, in_maps, *args):
    import os

    _enable_jit_cache()
    trace = bool(int(os.environ.get("BASS_KERNEL_PROFILE", "0")))
    r = run_bass_kernel_spmd(_prog(name, *args), in_maps, CORE_IDS, trace=trace)
    if trace:
        LAST_EXEC_NS.append((name, r.exec_time_ns, r.mean_exec_time_ns))
    return r.results


# --------------------------------------------------------------------------
# Host orchestration
# --------------------------------------------------------------------------

def _ulp_quarter(e64):
    """0.25 * (lower bound of the bf16 ulp at e), elementwise, float64."""
    a = np.abs(e64)
    a = np.where(a < 1e-30, 1e-30, a)
    return 0.25 * np.exp2(np.floor(np.log2(a)) - 8.0)


def kernel(inputs: np.ndarray, targets: np.ndarray) -> np.ndarray:
    import jax
    import jax.numpy as jnp

    x = np.ascontiguousarray(inputs[:, 0]).astype(np.float32, copy=False)
    y = np.asarray(targets)
    sig = y == 1

    LAST_EXEC_NS.clear()

    # ---- edges (host, bit-exact with the reference) -----------------------
    gmin = np.float32(x.min())
    gmax = np.float32(x.max())
    cpu = jax.devices("cpu")[0]
    with jax.default_device(cpu):
        edges = np.asarray(jnp.linspace(jnp.float32(gmin), jnp.float32(gmax), E))
    e_cmp = edges.astype(BF).astype(np.float32)          # bf16-grid thresholds
    e_act64 = e_cmp.astype(np.float64) + _ulp_quarter(e_cmp.astype(np.float64))
    e_act = e_act64.astype(np.float32)                   # strictly between grid pts

    # ---- class-compacted bf16 tiles --------------------------------------
    xb_all = x.astype(BF)
    xs_sig = xb_all[sig]
    xs_bg = xb_all[~sig]
    Ns_i, Nb_i = xs_sig.size, xs_bg.size
    cap_rows = N_CORES * HP
    F2 = (max(Ns_i, Nb_i) + cap_rows - 1) // cap_rows
    if F2 % 2:
        F2 += 1
    big_bf = BF(BIG)
    sig_pad = np.full(cap_rows * F2, big_bf, dtype=BF)
    sig_pad[:Ns_i] = xs_sig
    bg_pad = np.full(cap_rows * F2, big_bf, dtype=BF)
    bg_pad[:Nb_i] = xs_bg
    sig_tiles = sig_pad.reshape(N_CORES, HP, F2)
    bg_tiles = bg_pad.reshape(N_CORES, HP, F2)

    # edge -> slot assignment: A slots 0..N_A-1, B slots N_A..N_A+N_B-1 use
    # e_cmp; C slots use -e_act. Keep natural order (all e_act distinct from
    # e_cmp by construction; _ulp_quarter guards tiny magnitudes).
    ed_in = np.zeros((P, 2 * E), np.float32)
    ed_in[:, :E] = np.concatenate([e_cmp[:N_A], e_cmp[N_A : N_A + N_B],
                                   np.zeros(E - N_A - N_B, np.float32)])
    ed_in[:, E : E + N_C] = -e_act[N_A + N_B :]

    MROWS = 2 * N_B
    oh = np.zeros((P, N_B * MROWS), BF)
    for i in range(N_B):
        oh[0:HP, i * MROWS + 2 * i] = 1        # signal rows -> psum row 2i
        oh[HP:P, i * MROWS + 2 * i + 1] = 1    # background rows -> 2i+1

    res = _run(
        "counts",
        [
            {
                "xt": np.ascontiguousarray(
                    np.concatenate([sig_tiles[c], bg_tiles[c]], axis=0)
                ),
                "edges": ed_in,
                "oh": oh,
            }
            for c in CORE_IDS
        ],
        F2,
    )

    # ---- decode device counts (counts of xb <= e_cmp[k], per class) ------
    d_sig = np.zeros(E, np.float64)
    d_bg = np.zeros(E, np.float64)
    TOT_HALF = N_CORES * HP * F2
    for r in res:
        a = r["acca"].astype(np.float64)      # [P, N_A]
        c = r["accc"].astype(np.float64)      # [P, N_C]
        b = r["accb"].astype(np.float64)      # [MROWS, 1]
        d_sig[0:N_A] += a[0:HP].sum(axis=0)
        d_bg[0:N_A] += a[HP:P].sum(axis=0)
        d_sig[N_A : N_A + N_B] += b[0::2, 0]
        d_bg[N_A : N_A + N_B] += b[1::2, 0]
        # ACT: S = sum sign(x - e_act); le = (TOT - S)/2 per class half
        d_sig[N_A + N_B :] -= c[0:HP].sum(axis=0) / 2.0
        d_bg[N_A + N_B :] -= c[HP:P].sum(axis=0) / 2.0
    d_sig[N_A + N_B :] += TOT_HALF / 2.0
    d_bg[N_A + N_B :] += TOT_HALF / 2.0
    # pads count as "greater" in every path: subtract nothing for le counts.

    # ---- host repair: exact le/lt counts under fp32 semantics ------------
    h64 = (np.float64(gmax) - np.float64(gmin)) / N_BINS
    inv_h = np.float32(1.0 / h64) if h64 != 0 else np.float32(0.0)
    u = (x - gmin) * inv_h
    k0 = np.rint(u).astype(np.int64)
    tol = (6.0 / 128.0) * (np.abs(edges.astype(np.float64)) + 0.01)
    cand_mask = np.zeros(N, bool)
    for dk in (-1, 0, 1):
        kk = np.clip(k0 + dk, 0, E - 1)
        cand_mask |= np.abs(x.astype(np.float64) - edges[kk]) <= tol[kk]
    ci = np.flatnonzero(cand_mask)
    cx = x[ci]
    cxb = xb_all[ci].astype(np.float32)
    csig = sig[ci]
    ck0 = k0[ci]

    corr_le_sig = np.zeros(E, np.float64)
    corr_le_bg = np.zeros(E, np.float64)
    corr_lt_sig = np.zeros(E, np.float64)
    corr_lt_bg = np.zeros(E, np.float64)
    for dk in (-1, 0, 1):
        kr = ck0 + dk
        use = (kr >= 0) & (kr < E)
        kku = kr[use]
        dev = (cxb[use] <= e_cmp[kku]).astype(np.float64)
        t_le = (cx[use] <= edges[kku]).astype(np.float64)
        t_lt = (cx[use] < edges[kku]).astype(np.float64)
        s_u = csig[use]
        np.add.at(corr_le_sig, kku[s_u], (t_le - dev)[s_u])
        np.add.at(corr_le_bg, kku[~s_u], (t_le - dev)[~s_u])
        np.add.at(corr_lt_sig, kku[s_u], (t_lt - dev)[s_u])
        np.add.at(corr_lt_bg, kku[~s_u], (t_lt - dev)[~s_u])

    ns_le = (d_sig + corr_le_sig).astype(np.float32)
    nb_le = (d_bg + corr_le_bg).astype(np.float32)
    ns_lt = (d_sig + corr_lt_sig).astype(np.float32)
    nb_lt = (d_bg + corr_lt_bg).astype(np.float32)

    # ---- replicate the reference's pair search bit-exactly ----------------
    with jax.default_device(cpu):
        ns_le_j = jnp.asarray(ns_le)
        ns_lt_j = jnp.asarray(ns_lt)
        nb_le_j = jnp.asarray(nb_le)
        nb_lt_j = jnp.asarray(nb_lt)
        n_f = jnp.float32(N)
        Ns = ns_le_j[-1]
        Nb = n_f - Ns

        hist0 = nb_le_j[1:] - nb_lt_j[:-1]
        hist1 = ns_le_j[1:] - ns_lt_j[:-1]

        gt0 = hist0 > hist1
        cand0 = jnp.logical_xor(gt0[:-1], gt0[1:]) & (hist0[:-1] > 0)
        gt1 = hist1 > hist0
        cand1 = jnp.logical_xor(gt1[:-1], gt1[1:]) & (hist1[:-1] > 0)
        mask = jnp.zeros((E,), bool).at[1:N_BINS].set(cand0 | cand1)
        cnt = jnp.sum(mask)
        mask = mask.at[-1].set(mask[-1] | (cnt == 1))

        a_c = -jnp.log1p(jnp.float32(-EPS))
        b_c = -jnp.log(jnp.float32(EPS))

        def bce(correct):
            return ((n_f - correct) * b_c + correct * a_c) / n_f

        c0 = ns_le_j + (Nb - nb_le_j)
        c1 = (Ns - ns_lt_j) + nb_lt_j
        c2 = (ns_le_j[None, :] - ns_lt_j[:, None]) + Nb - (
            nb_le_j[None, :] - nb_lt_j[:, None]
        )
        c3 = ns_le_j[:, None] + (Ns - ns_lt_j[None, :]) + (
            nb_le_j[None, :] - nb_lt_j[:, None]
        )

        L = jnp.stack(
            [
                jnp.broadcast_to(bce(c0)[:, None], (E, E)),
                jnp.broadcast_to(bce(c1)[:, None], (E, E)),
                bce(c2),
                bce(c3),
            ]
        )
        per_pair_min = jnp.min(L, axis=0)
        per_pair_case = jnp.argmin(L, axis=0)

        idxs = jnp.arange(E)
        valid = mask[:, None] & mask[None, :] & (idxs[:, None] < idxs[None, :])
        flat = jnp.argmin(jnp.where(valid, per_pair_min, jnp.inf))
        i = int(flat) // E
        j = int(flat) % E
        lower = np.float32(edges[i])
        upper = np.float32(edges[j])
        case = int(per_pair_case[i, j])

    # ---- L2: predicate on bf16 grid (original order) ----------------------
    lo_cmp = np.float32(BF(lower))
    up_cmp = np.float32(BF(upper))
    prm = np.zeros((P, 8), np.float32)
    prm[:, 0] = lo_cmp
    prm[:, 1] = up_cmp

    res3 = _run(
        f"pred{case}",
        [
            {"x": xb_all[c * CORE_N : c * CORE_N + DEV_N], "prm": prm}
            for c in CORE_IDS
        ],
    )

    out = np.empty(N, np.int32)
    for c in CORE_IDS:
        out[c * CORE_N : c * CORE_N + DEV_N] = (
            res3[c]["pred"].reshape(-1).astype(np.float32) != 0.0
        )
        # layout tail: exact on host
        tx = x[c * CORE_N + DEV_N : (c + 1) * CORE_N]
        if case == 0:
            tp = tx <= lower
        elif case == 1:
            tp = tx >= lower
        elif case == 2:
            tp = (tx >= lower) & (tx <= upper)
        else:
            tp = (tx <= lower) | (tx >= upper)
        out[c * CORE_N + DEV_N : (c + 1) * CORE_N] = tp.astype(np.int32)

    # repair events within bf16 rounding of the cuts (exact fp32 predicate)
    tol_lo = 6.0 / 128.0 * (abs(float(lower)) + 0.01)
    tol_up = 6.0 / 128.0 * (abs(float(upper)) + 0.01)
    rep = (np.abs(x - lower) <= tol_lo) | (np.abs(x - upper) <= tol_up)
    ri = np.flatnonzero(rep)
    rx = x[ri]
    if case == 0:
        rp = rx <= lower
    elif case == 1:
        rp = rx >= lower
    elif case == 2:
        rp = (rx >= lower) & (rx <= upper)
    else:
        rp = (rx <= lower) | (rx >= upper)
    out[ri] = rp.astype(np.int32)
    return out


# revision 5
# speedup vs baseline: 2.0435x; 1.0337x over previous
"""Trainium2 Bass kernel for nn_CutLayer (histogram_binning).

Two device launches over 8 cores (data-parallel on events):

L1 "counts": per-core class-compacted bf16 tile [128, F2] (rows 0-63 =
  signal events, rows 64-127 = background, pads = +BIG). 51 bf16-grid
  edge thresholds are counted by three engine paths running concurrently:
    - PE path (N_B edges): DVE plain tensor_scalar compare at 4x rate
      (bf16) into ping-pong buffers; PE reduces each compare tile with a
      one-hot stationary matmul into PSUM rows [2 per edge], accumulated
      across 512-column chunks; one final DVE reduce -> [2*N_B, 1].
    - DVE-accum path (N_A edges): fused tensor_scalar compare+accum (1x).
    - ACT path (N_C edges): Sign activation with bias strictly between
      bf16 grid points (no sign(0) ties) + accumulator.
  Host converts to exact fp32-semantics counts (le and lt) by correcting
  a small candidate set of events within a few bf16 ulps of each edge,
  then replicates the reference's pair search bit-exactly (eager CPU jax)
  to produce (lower, upper, case).

L2 "pred": case-specialized bf16 compare(s) on the original-order
  events; host flips the few events within bf16 rounding of the chosen
  cuts and handles the 512-event layout tail exactly.

Host work is O(N) numpy prep/fixup only: min/max, bf16 casts, class
compaction, candidate repair, tiny 51x51 pair search.
"""

from contextlib import ExitStack

import numpy as np
import ml_dtypes

import concourse.bass as bass
import concourse.mybir as mybir
from concourse.bass_utils import run_bass_kernel_spmd

N = 8_000_000
N_CORES = 8
CORE_N = N // N_CORES            # 1_000_000
P = 128
HP = 64                          # rows per class in the counts tile
N_BINS = 50
E = N_BINS + 1                   # 51 edges
EPS = 1e-7
BIG = np.float32(1.0e30)         # bf16-exact sentinel above every edge

# pred layout (original order)
FP = 7812
DEV_N = P * FP                   # 999_936 device events/core for pred

# counts engine split (sums to E)
N_B = 20                         # DVE compare -> PE matmul reduce
N_A = 11                         # DVE fused compare+accum
N_C = 20                         # ACT sign path
CH = 512                         # psum chunk columns

FP32 = mybir.dt.float32
BF16 = mybir.dt.bfloat16
I32 = mybir.dt.int32
AX = mybir.AxisListType
OP = mybir.AluOpType
ACT = mybir.ActivationFunctionType

CORE_IDS = list(range(N_CORES))
BF = ml_dtypes.bfloat16


# --------------------------------------------------------------------------
# Bass programs
# --------------------------------------------------------------------------

def _build_counts_v2(F2: int):
    nchunks = (F2 + CH - 1) // CH
    MROWS = 2 * N_B
    nc = bass.Bass()
    xt_in = nc.declare_dram_parameter("xt", [P, F2], BF16, isOutput=False)
    ed = nc.declare_dram_parameter("edges", [P, 2 * E], FP32, isOutput=False)
    oh_in = nc.declare_dram_parameter("oh", [P, N_B * MROWS], BF16,
                                      isOutput=False)
    acca_o = nc.declare_dram_parameter("acca", [P, N_A], FP32, isOutput=True)
    accc_o = nc.declare_dram_parameter("accc", [P, N_C], FP32, isOutput=True)
    accb_o = nc.declare_dram_parameter("accb", [MROWS, 1], FP32, isOutput=True)
    with ExitStack() as es:
        ec = es.enter_context
        xt = ec(nc.sbuf_tensor([P, F2], BF16))
        cb = [ec(nc.sbuf_tensor(f"cb{i}", [P, F2], BF16)) for i in range(2)]
        scra = ec(nc.sbuf_tensor([P, F2], BF16))
        scrc = ec(nc.sbuf_tensor([P, F2], BF16))
        edt = ec(nc.sbuf_tensor([P, 2 * E], FP32))
        oht = ec(nc.sbuf_tensor([P, N_B * MROWS], BF16))
        acca = ec(nc.sbuf_tensor([P, N_A], FP32))
        accc = ec(nc.sbuf_tensor([P, N_C], FP32))
        accb = ec(nc.sbuf_tensor([MROWS, 1], FP32))
        ps = ec(nc.psum_tensor([MROWS, CH], FP32))
        dse = ec(nc.semaphore("dse"))
        dsx0 = ec(nc.semaphore("dsx0"))
        dsx1 = ec(nc.semaphore("dsx1"))
        vprod = ec(nc.semaphore("vprod"))
        tcons = ec(nc.semaphore("tcons"))
        adone = ec(nc.semaphore("adone"))
        cdone = ec(nc.semaphore("cdone"))
        bdone = ec(nc.semaphore("bdone"))
        dso = ec(nc.semaphore("dso"))
        block = ec(nc.Block())

        H2 = F2 // 2

        @block.sync
        def _(sync):
            sync.dma_start(edt[:], ed[:]).then_inc(dse, 16)
            sync.dma_start(oht[:], oh_in[:]).then_inc(dse, 16)
            sync.dma_start(xt[:, 0:H2], xt_in[:, 0:H2]).then_inc(dsx0, 16)
            sync.wait_ge(adone, 1)
            sync.dma_start(acca_o[:], acca[:]).then_inc(dso, 16)
            sync.wait_ge(cdone, 1)
            sync.dma_start(accc_o[:], accc[:]).then_inc(dso, 16)
            sync.wait_ge(bdone, 1)
            sync.dma_start(accb_o[:], accb[:]).then_inc(dso, 16)
            sync.wait_ge(dso, 48)

        @block.scalar
        def _(scalar):
            scalar.dma_start(xt[:, H2:F2], xt_in[:, H2:F2]).then_inc(dsx1, 16)
            scalar.wait_ge(dse, 32)
            scalar.wait_ge(dsx0, 16)
            scalar.wait_ge(dsx1, 16)
            for j in range(N_C):
                ins = scalar.activation(
                    scrc[:], xt[:], ACT.Sign, bias=edt[:, E + j : E + j + 1],
                    scale=1.0, accum_out=accc[:, j : j + 1],
                )
                if j == N_C - 1:
                    ins.then_inc(cdone, 1)

        @block.vector
        def _(vector):
            vector.wait_ge(dse, 32)
            vector.wait_ge(dsx0, 16)
            vector.wait_ge(dsx1, 16)
            na = 0
            last_a_ins = None
            for i in range(N_B):
                if i >= 2:
                    vector.wait_ge(tcons, i - 1)
                vector.tensor_scalar(
                    cb[i % 2][:], xt[:], edt[:, N_A + i : N_A + i + 1], None,
                    OP.is_le,
                ).then_inc(vprod, 1)
                if (i % 2 == 1) and na < N_A:
                    last_a_ins = vector.tensor_scalar(
                        scra[:], xt[:], edt[:, na : na + 1], 0.0,
                        OP.is_le, OP.add, accum_out=acca[:, na : na + 1],
                    )
                    na += 1
            while na < N_A:
                last_a_ins = vector.tensor_scalar(
                    scra[:], xt[:], edt[:, na : na + 1], 0.0,
                    OP.is_le, OP.add, accum_out=acca[:, na : na + 1],
                )
                na += 1
            if last_a_ins is not None:
                last_a_ins.then_inc(adone, 1)
            vector.wait_ge(tcons, N_B)
            vector.tensor_reduce(
                accb[:, 0:1], ps[:, 0:CH], axis=AX.X, op=OP.add
            ).then_inc(bdone, 1)

        @block.tensor
        def _(tensor):
            first = True
            for i in range(N_B):
                tensor.wait_ge(vprod, i + 1)
                w = oht[:, i * MROWS : (i + 1) * MROWS]
                for c in range(nchunks):
                    c0 = c * CH
                    c1 = min(F2, c0 + CH)
                    ins = tensor.matmul(
                        ps[:, 0 : c1 - c0], w, cb[i % 2][:, c0:c1],
                        start=first,
                        stop=(i == N_B - 1 and c == nchunks - 1),
                        skip_group_check=True,
                    )
                    if c > 0:
                        ins.ins.ldweights = False
                    first = False
                ins.then_inc(tcons, 1)
    return nc


def _build_pred(case: int):
    """Case-specialized predicate on bf16 events (original order):
    0: x <= lo ; 1: x >= lo ; 2: (x >= lo) & (x <= up) ;
    3: (x <= lo) | (x >= up)  (disjoint -> add)
    """
    nc = bass.Bass()
    x = nc.declare_dram_parameter("x", [DEV_N], BF16, isOutput=False)
    pr = nc.declare_dram_parameter("prm", [P, 8], FP32, isOutput=False)
    out = nc.declare_dram_parameter("pred", [DEV_N], BF16, isOutput=True)
    HF = FP // 2
    with (
        nc.sbuf_tensor([P, FP], BF16) as xt,
        nc.sbuf_tensor([P, FP], BF16) as t,
        nc.sbuf_tensor([P, FP], BF16) as s,
        nc.sbuf_tensor([P, FP], BF16) as pi,
        nc.sbuf_tensor([P, 8], FP32) as prm,
        nc.semaphore("d0") as d0,
        nc.semaphore("d1") as d1,
        nc.semaphore("csem") as csem,
        nc.semaphore("dso") as dso,
        nc.Block() as block,
    ):
        xv = x[:].rearrange("(p f) -> p f", p=P)
        ov = out[:].rearrange("(p f) -> p f", p=P)

        @block.sync
        def _(sync):
            sync.dma_start(prm[:], pr[:]).then_inc(d0, 16)
            sync.dma_start(xt[:, 0:HF], xv[:, 0:HF]).then_inc(d0, 16)
            sync.wait_ge(csem, 1)
            sync.dma_start(ov[:, 0:HF], pi[:, 0:HF]).then_inc(dso, 16)
            sync.wait_ge(csem, 2)
            sync.dma_start(ov[:, HF:FP], pi[:, HF:FP]).then_inc(dso, 16)
            sync.wait_ge(dso, 32)

        @block.scalar
        def _(scalar):
            scalar.dma_start(xt[:, HF:FP], xv[:, HF:FP]).then_inc(d1, 16)

        @block.vector
        def _(vector):
            lo = prm[:, 0:1]
            up = prm[:, 1:2]
            vector.wait_ge(d0, 32)
            for h, sem in ((0, d0), (1, d1)):
                if h == 1:
                    vector.wait_ge(d1, 16)
                sl = slice(0, HF) if h == 0 else slice(HF, FP)
                if case == 0:
                    vector.tensor_scalar(
                        pi[:, sl], xt[:, sl], lo, None, OP.is_le
                    ).then_inc(csem, 1)
                elif case == 1:
                    vector.tensor_scalar(
                        pi[:, sl], xt[:, sl], lo, None, OP.is_ge
                    ).then_inc(csem, 1)
                elif case == 2:
                    vector.tensor_scalar(t[:, sl], xt[:, sl], up, None,
                                         OP.is_le)
                    vector.tensor_scalar(s[:, sl], xt[:, sl], lo, None,
                                         OP.is_ge)
                    vector.tensor_tensor(
                        pi[:, sl], s[:, sl], t[:, sl], OP.mult
                    ).then_inc(csem, 1)
                else:
                    vector.tensor_scalar(t[:, sl], xt[:, sl], up, None,
                                         OP.is_ge)
                    vector.tensor_scalar(s[:, sl], xt[:, sl], lo, None,
                                         OP.is_le)
                    vector.tensor_tensor(
                        pi[:, sl], s[:, sl], t[:, sl], OP.add
                    ).then_inc(csem, 1)
    return nc


_PROGRAMS: dict = {}


def _prog(name, *args):
    key = (name, args)
    if key not in _PROGRAMS:
        if name == "counts":
            _PROGRAMS[key] = _build_counts_v2(*args)
        else:
            _PROGRAMS[key] = _build_pred(int(name[4:]))
    return _PROGRAMS[key]


LAST_EXEC_NS: list = []
_CACHE_SET = False


def _enable_jit_cache():
    global _CACHE_SET
    if _CACHE_SET:
        return
    _CACHE_SET = True
    try:
        import jax

        jax.config.update("jax_compilation_cache_dir", "/tmp/jax_bass_cache")
        jax.config.update("jax_persistent_cache_min_compile_time_secs", 1.0)
        jax.config.update("jax_persistent_cache_min_entry_size_bytes", 0)
    except Exception:
        pass


def _run(name, in_maps, *args):
    import os

    _enable_jit_cache()
    trace = bool(int(os.environ.get("BASS_KERNEL_PROFILE", "0")))
    r = run_bass_kernel_spmd(_prog(name, *args), in_maps, CORE_IDS, trace=trace)
    if trace:
        LAST_EXEC_NS.append((name, r.exec_time_ns, r.mean_exec_time_ns))
    return r.results


# --------------------------------------------------------------------------
# Host orchestration
# --------------------------------------------------------------------------

def _ulp_quarter(e64):
    """0.25 * (lower bound of the bf16 ulp at e), elementwise, float64."""
    a = np.abs(e64)
    a = np.where(a < 1e-30, 1e-30, a)
    return 0.25 * np.exp2(np.floor(np.log2(a)) - 8.0)


def kernel(inputs: np.ndarray, targets: np.ndarray) -> np.ndarray:
    import jax
    import jax.numpy as jnp

    x = np.ascontiguousarray(inputs[:, 0]).astype(np.float32, copy=False)
    y = np.asarray(targets)
    sig = y == 1

    LAST_EXEC_NS.clear()

    # ---- edges (host, bit-exact with the reference) -----------------------
    gmin = np.float32(x.min())
    gmax = np.float32(x.max())
    cpu = jax.devices("cpu")[0]
    with jax.default_device(cpu):
        edges = np.asarray(jnp.linspace(jnp.float32(gmin), jnp.float32(gmax), E))
    e_cmp = edges.astype(BF).astype(np.float32)          # bf16-grid thresholds
    e_act64 = e_cmp.astype(np.float64) + _ulp_quarter(e_cmp.astype(np.float64))
    e_act = e_act64.astype(np.float32)                   # strictly between grid pts

    # ---- class-compacted bf16 tiles --------------------------------------
    xb_all = x.astype(BF)
    xs_sig = xb_all[sig]
    xs_bg = xb_all[~sig]
    Ns_i, Nb_i = xs_sig.size, xs_bg.size
    cap_rows = N_CORES * HP
    F2 = (max(Ns_i, Nb_i) + cap_rows - 1) // cap_rows
    if F2 % 2:
        F2 += 1
    big_bf = BF(BIG)
    sig_pad = np.full(cap_rows * F2, big_bf, dtype=BF)
    sig_pad[:Ns_i] = xs_sig
    bg_pad = np.full(cap_rows * F2, big_bf, dtype=BF)
    bg_pad[:Nb_i] = xs_bg
    sig_tiles = sig_pad.reshape(N_CORES, HP, F2)
    bg_tiles = bg_pad.reshape(N_CORES, HP, F2)

    # edge -> slot assignment: A slots 0..N_A-1, B slots N_A..N_A+N_B-1 use
    # e_cmp; C slots use -e_act. Keep natural order (all e_act distinct from
    # e_cmp by construction; _ulp_quarter guards tiny magnitudes).
    ed_in = np.zeros((P, 2 * E), np.float32)
    ed_in[:, :E] = np.concatenate([e_cmp[:N_A], e_cmp[N_A : N_A + N_B],
                                   np.zeros(E - N_A - N_B, np.float32)])
    ed_in[:, E : E + N_C] = -e_act[N_A + N_B :]

    MROWS = 2 * N_B
    oh = np.zeros((P, N_B * MROWS), BF)
    for i in range(N_B):
        oh[0:HP, i * MROWS + 2 * i] = 1        # signal rows -> psum row 2i
        oh[HP:P, i * MROWS + 2 * i + 1] = 1    # background rows -> 2i+1

    res = _run(
        "counts",
        [
            {
                "xt": np.ascontiguousarray(
                    np.concatenate([sig_tiles[c], bg_tiles[c]], axis=0)
                ),
                "edges": ed_in,
                "oh": oh,
            }
            for c in CORE_IDS
        ],
        F2,
    )

    # ---- decode device counts (counts of xb <= e_cmp[k], per class) ------
    d_sig = np.zeros(E, np.float64)
    d_bg = np.zeros(E, np.float64)
    TOT_HALF = N_CORES * HP * F2
    for r in res:
        a = r["acca"].astype(np.float64)      # [P, N_A]
        c = r["accc"].astype(np.float64)      # [P, N_C]
        b = r["accb"].astype(np.float64)      # [MROWS, 1]
        d_sig[0:N_A] += a[0:HP].sum(axis=0)
        d_bg[0:N_A] += a[HP:P].sum(axis=0)
        d_sig[N_A : N_A + N_B] += b[0::2, 0]
        d_bg[N_A : N_A + N_B] += b[1::2, 0]
        # ACT: S = sum sign(x - e_act); le = (TOT - S)/2 per class half
        d_sig[N_A + N_B :] -= c[0:HP].sum(axis=0) / 2.0
        d_bg[N_A + N_B :] -= c[HP:P].sum(axis=0) / 2.0
    d_sig[N_A + N_B :] += TOT_HALF / 2.0
    d_bg[N_A + N_B :] += TOT_HALF / 2.0
    # pads count as "greater" in every path: subtract nothing for le counts.

    # ---- host repair: exact le/lt counts under fp32 semantics ------------
    h64 = (np.float64(gmax) - np.float64(gmin)) / N_BINS
    inv_h = np.float32(1.0 / h64) if h64 != 0 else np.float32(0.0)
    u = (x - gmin) * inv_h
    k0 = np.rint(u).astype(np.int64)
    tol = (6.0 / 128.0) * (np.abs(edges.astype(np.float64)) + 0.01)
    cand_mask = np.zeros(N, bool)
    for dk in (-1, 0, 1):
        kk = np.clip(k0 + dk, 0, E - 1)
        cand_mask |= np.abs(x.astype(np.float64) - edges[kk]) <= tol[kk]
    ci = np.flatnonzero(cand_mask)
    cx = x[ci]
    cxb = xb_all[ci].astype(np.float32)
    csig = sig[ci]
    ck0 = k0[ci]

    corr_le_sig = np.zeros(E, np.float64)
    corr_le_bg = np.zeros(E, np.float64)
    corr_lt_sig = np.zeros(E, np.float64)
    corr_lt_bg = np.zeros(E, np.float64)
    for dk in (-1, 0, 1):
        kr = ck0 + dk
        use = (kr >= 0) & (kr < E)
        kku = kr[use]
        dev = (cxb[use] <= e_cmp[kku]).astype(np.float64)
        t_le = (cx[use] <= edges[kku]).astype(np.float64)
        t_lt = (cx[use] < edges[kku]).astype(np.float64)
        s_u = csig[use]
        np.add.at(corr_le_sig, kku[s_u], (t_le - dev)[s_u])
        np.add.at(corr_le_bg, kku[~s_u], (t_le - dev)[~s_u])
        np.add.at(corr_lt_sig, kku[s_u], (t_lt - dev)[s_u])
        np.add.at(corr_lt_bg, kku[~s_u], (t_lt - dev)[~s_u])

    ns_le = (d_sig + corr_le_sig).astype(np.float32)
    nb_le = (d_bg + corr_le_bg).astype(np.float32)
    ns_lt = (d_sig + corr_lt_sig).astype(np.float32)
    nb_lt = (d_bg + corr_lt_bg).astype(np.float32)

    # ---- replicate the reference's pair search bit-exactly ----------------
    with jax.default_device(cpu):
        ns_le_j = jnp.asarray(ns_le)
        ns_lt_j = jnp.asarray(ns_lt)
        nb_le_j = jnp.asarray(nb_le)
        nb_lt_j = jnp.asarray(nb_lt)
        n_f = jnp.float32(N)
        Ns = ns_le_j[-1]
        Nb = n_f - Ns

        hist0 = nb_le_j[1:] - nb_lt_j[:-1]
        hist1 = ns_le_j[1:] - ns_lt_j[:-1]

        gt0 = hist0 > hist1
        cand0 = jnp.logical_xor(gt0[:-1], gt0[1:]) & (hist0[:-1] > 0)
        gt1 = hist1 > hist0
        cand1 = jnp.logical_xor(gt1[:-1], gt1[1:]) & (hist1[:-1] > 0)
        mask = jnp.zeros((E,), bool).at[1:N_BINS].set(cand0 | cand1)
        cnt = jnp.sum(mask)
        mask = mask.at[-1].set(mask[-1] | (cnt == 1))

        a_c = -jnp.log1p(jnp.float32(-EPS))
        b_c = -jnp.log(jnp.float32(EPS))

        def bce(correct):
            return ((n_f - correct) * b_c + correct * a_c) / n_f

        c0 = ns_le_j + (Nb - nb_le_j)
        c1 = (Ns - ns_lt_j) + nb_lt_j
        c2 = (ns_le_j[None, :] - ns_lt_j[:, None]) + Nb - (
            nb_le_j[None, :] - nb_lt_j[:, None]
        )
        c3 = ns_le_j[:, None] + (Ns - ns_lt_j[None, :]) + (
            nb_le_j[None, :] - nb_lt_j[:, None]
        )

        L = jnp.stack(
            [
                jnp.broadcast_to(bce(c0)[:, None], (E, E)),
                jnp.broadcast_to(bce(c1)[:, None], (E, E)),
                bce(c2),
                bce(c3),
            ]
        )
        per_pair_min = jnp.min(L, axis=0)
        per_pair_case = jnp.argmin(L, axis=0)

        idxs = jnp.arange(E)
        valid = mask[:, None] & mask[None, :] & (idxs[:, None] < idxs[None, :])
        flat = jnp.argmin(jnp.where(valid, per_pair_min, jnp.inf))
        i = int(flat) // E
        j = int(flat) % E
        lower = np.float32(edges[i])
        upper = np.float32(edges[j])
        case = int(per_pair_case[i, j])

    # ---- L2: predicate on bf16 grid (original order) ----------------------
    lo_cmp = np.float32(BF(lower))
    up_cmp = np.float32(BF(upper))
    prm = np.zeros((P, 8), np.float32)
    prm[:, 0] = lo_cmp
    prm[:, 1] = up_cmp

    res3 = _run(
        f"pred{case}",
        [
            {"x": xb_all[c * CORE_N : c * CORE_N + DEV_N], "prm": prm}
            for c in CORE_IDS
        ],
    )

    out = np.empty(N, np.int32)
    for c in CORE_IDS:
        out[c * CORE_N : c * CORE_N + DEV_N] = (
            res3[c]["pred"].reshape(-1).astype(np.float32) != 0.0
        )
        # layout tail: exact on host
        tx = x[c * CORE_N + DEV_N : (c + 1) * CORE_N]
        if case == 0:
            tp = tx <= lower
        elif case == 1:
            tp = tx >= lower
        elif case == 2:
            tp = (tx >= lower) & (tx <= upper)
        else:
            tp = (tx <= lower) | (tx >= upper)
        out[c * CORE_N + DEV_N : (c + 1) * CORE_N] = tp.astype(np.int32)

    # repair events within bf16 rounding of the cuts (exact fp32 predicate)
    tol_lo = 6.0 / 128.0 * (abs(float(lower)) + 0.01)
    tol_up = 6.0 / 128.0 * (abs(float(upper)) + 0.01)
    rep = (np.abs(x - lower) <= tol_lo) | (np.abs(x - upper) <= tol_up)
    ri = np.flatnonzero(rep)
    rx = x[ri]
    if case == 0:
        rp = rx <= lower
    elif case == 1:
        rp = rx >= lower
    elif case == 2:
        rp = (rx >= lower) & (rx <= upper)
    else:
        rp = (rx <= lower) | (rx >= upper)
    out[ri] = rp.astype(np.int32)
    return out
